# revision 28
# baseline (speedup 1.0000x reference)
"""PixelMixBlock TRN2 kernel: per-image attention (q=Wq@x, k=Wq@x[idx], v=Wv@x[idx]),
softmax over k via exp + PE column-sum, mask=pw@v, bilinear 8x upsample as A@M@A^T,
sigmoid on ACT. Data-parallel over batch N across 8 cores (8 images/core).

Pipeline: input DMA / q,k convs / score+exp+num-den streamed per image so PE and
ACT overlap from the start; per-image DRAM roundtrip de-interleaves num/den; the
upsample phase runs z = M^T A^T then up = (z_half)^T A^T (no PE transpose)."""
import sys

for _p in ("/opt/trn_rl_repo",):
    if _p not in sys.path:
        sys.path.insert(0, _p)

import numpy as np
import ml_dtypes

import concourse.bacc as bacc
import concourse.mybir as mybir
import concourse.tile as tile
from concourse.bass_utils import run_bass_kernel_spmd

AF = mybir.ActivationFunctionType
ALU = mybir.AluOpType
BF16 = mybir.dt.bfloat16
F32 = mybir.dt.float32

PER = 8          # images per core
NCORES = 8
HWN = 784        # 28*28
INTER = 128
NJ, KT = 7, 112  # k-tiles of the 784 key positions
HO = 224         # 28 * 8
SCALE = float(1.0 / np.sqrt(128.0))
# const pack layout (bf16 plane): wq0 | wq1 | ov[7]x16 | bmat
OV0 = 256
BM0 = OV0 + 16 * NJ
CBW = BM0 + HO

_NC_CACHE = {}


def _bilinear_matrix():
    # jax.image.resize bilinear (align_corners=False): triangle kernel, edge-renormalized
    A = np.zeros((HO, 28), np.float64)
    for i in range(HO):
        s = (i + 0.5) / 8.0 - 0.5
        for j in range(28):
            A[i, j] = max(0.0, 1.0 - abs(s - j))
        A[i] /= A[i].sum()
    return A.astype(np.float32)


def _build_nc():
    nc = bacc.Bacc("TRN2", target_bir_lowering=False, debug=False, num_devices=NCORES)
    xin_d = nc.dram_tensor("xin", [PER, 2, 128, HWN], BF16, kind="ExternalInput").ap()
    xpair_d = nc.dram_tensor("xpair", [PER, 2, 128, HWN], BF16, kind="ExternalInput").ap()
    cbf_d = nc.dram_tensor("cbf", [128, CBW], BF16, kind="ExternalInput").ap()
    cf32_d = nc.dram_tensor("cf32", [128, 2], F32, kind="ExternalInput").ap()
    out_d = nc.dram_tensor("out", [PER, 2, HO, HO], F32, kind="ExternalOutput").ap()
    nd_d = nc.dram_tensor("ndscr", [2, PER * HWN], F32)

    with tile.TileContext(nc) as tc:
        _emit(nc, tc, xin_d, xpair_d, cbf_d, cf32_d, out_d, nd_d)
    nc.compile()
    return nc


def _emit(nc, tc, xin_d, xpair_d, cbf_d, cf32_d, out_d, nd_d):
    from contextlib import ExitStack
    with ExitStack() as ctx:
        consts = ctx.enter_context(tc.tile_pool(name="consts", bufs=1))
        data = ctx.enter_context(tc.tile_pool(name="data", bufs=1))
        xpool = ctx.enter_context(tc.tile_pool(name="xpool", bufs=8))
        epool = ctx.enter_context(tc.tile_pool(name="epool", bufs=3))
        spool = ctx.enter_context(tc.tile_pool(name="spool", bufs=2))
        opool = ctx.enter_context(tc.tile_pool(name="opool", bufs=3))

        cbf = consts.tile([128, CBW], BF16, tag="cbf", name="cbf")
        cf32 = consts.tile([128, 2], F32, tag="cf32", name="cf32")
        nc.sync.dma_start(cbf[:], cbf_d[:])
        wq = [cbf[:, 0:128], cbf[:, 128:256]]
        bmat = cbf[0:28, BM0:BM0 + HO]
        bqap = cf32[:, 0:1]
        bvap = cf32[0:112, 1:2]

        qs = [data.tile([128, HWN], BF16, tag=f"qs{i}", name=f"qs{i}") for i in range(PER)]
        ks = [data.tile([128, HWN], BF16, tag=f"ks{i}", name=f"ks{i}") for i in range(PER)]
        den28 = data.tile([28, HO], F32, tag="den28", name="den28")
        num28 = data.tile([28, HO], F32, tag="num28", name="num28")
        rcp28 = data.tile([28, HO], F32, tag="rcp28", name="rcp28")
        mask28 = data.tile([28, HO], BF16, tag="mask28", name="mask28")
        nd_sb = data.tile([2, PER * HWN], F32, tag="ndsb", name="ndsb")

        xtiles = [None] * PER

        def load(i):
            xt = xpool.tile([128, 2 * HWN], BF16, tag="x", name=f"xt{i}")
            xp = xpool.tile([128, 2 * HWN], BF16, tag="x", name=f"xp{i}")
            nc.sync.dma_start(xp[:].rearrange("p (t w) -> p t w", t=2),
                              xpair_d[i].rearrange("t p w -> p t w"))
            nc.sync.dma_start(xt[:].rearrange("p (t w) -> p t w", t=2),
                              xin_d[i].rearrange("t p w -> p t w"))
            xtiles[i] = (xt, xp)

        # PSUM: apsum 2 banks + cpsum 2x2 banks + ndpsum 2 banks = 8
        with tc.tile_pool(name="apsum", bufs=2, space="PSUM") as apsum, \
             tc.tile_pool(name="cpsum", bufs=2, space="PSUM") as cpsum, \
             tc.tile_pool(name="ndpsum", bufs=1, space="PSUM") as ndpsum:

            def conv_step(i, k):
                dst, sel, h = ((ks, 1, 0), (qs, 0, 0), (qs, 0, 1), (ks, 1, 1))[k]
                src = xtiles[i][sel]
                cs = slice(h * 392, (h + 1) * 392)
                pt = apsum.tile([128, 392], F32, tag="cv", name=f"cv{i}_{k}")
                for t in range(2):
                    s0 = t * HWN + h * 392
                    nc.tensor.matmul(pt[:], wq[t], src[:, s0:s0 + 392],
                                     start=(t == 0), stop=(t == 1))
                nc.vector.tensor_scalar(dst[i][:, cs], pt[:], bqap, None, ALU.add)

            nds = [None] * PER

            def emit_batch(b):
                b0, b1 = b * 4 * HWN, (b + 1) * 4 * HWN
                bc = slice(b * 112, (b + 1) * 112)
                nc.sync.dma_start(nd_d[:, b0:b1], nd_sb[0:2, b0:b1])
                nc.sync.dma_start(den28[:, bc].rearrange("h (i w) -> h i w", i=4, w=28),
                                  nd_d[0, b0:b1].rearrange("(i h w) -> h i w", i=4, h=28))
                nc.sync.dma_start(num28[:, bc].rearrange("h (i w) -> h i w", i=4, w=28),
                                  nd_d[1, b0:b1].rearrange("(i h w) -> h i w", i=4, h=28))
                nc.vector.reciprocal(rcp28[:, bc], den28[:, bc])
                nc.vector.tensor_tensor(mask28[:, bc], num28[:, bc], rcp28[:, bc], ALU.mult)

            def emit_nd(pend):
                i, j, e = pend
                if j == 0:
                    nds[i] = ndpsum.tile([2, 1024], F32, tag="nd", name=f"nd{i}")
                nd = nds[i]
                st, sp = (j == 0), (j == NJ - 1)
                lj = cbf[0:112, OV0 + 16 * j + 2 * i: OV0 + 16 * j + 2 * i + 2]
                nc.tensor.matmul(nd[:, 0:392], lj, e[:, 0:392], start=st, stop=sp)
                nc.tensor.matmul(nd[:, 512:904], lj, e[:, 392:784], start=st, stop=sp)
                if sp:
                    base = i * HWN
                    dstap = nd_sb[0:2, base:base + HWN].rearrange("p (g w) -> p g w", g=2, w=392)
                    srcap = nd[:].rearrange("p (g w) -> p g w", g=2, w=512)[:, :, 0:392]
                    nc.vector.tensor_copy(out=dstap, in_=srcap)
                    if i == 3:
                        emit_batch(0)
                    elif i == PER - 1:
                        emit_batch(1)

            load(0)
            nc.sync.dma_start(cf32[:], cf32_d[:])
            load(1)
            for k in range(4):
                conv_step(0, k)
            pend = None
            for i in range(PER):
                for j in range(NJ):
                    sc = cpsum.tile([112, 1024], F32, tag="sc", name=f"sc{i}_{j}")
                    e = epool.tile([112, HWN], BF16, tag="e", name=f"e{i}_{j}")
                    ksl = ks[i][:, j * KT:(j + 1) * KT]
                    nc.tensor.matmul(sc[:, 0:392], ksl, qs[i][:, 0:392], start=True, stop=True)
                    nc.tensor.matmul(sc[:, 512:904], ksl, qs[i][:, 392:784], start=True, stop=True)
                    if pend is not None:
                        emit_nd(pend)
                    if j == 0 and i + 2 < PER:
                        load(i + 2)
                    if 1 <= j <= 4 and i + 1 < PER:
                        conv_step(i + 1, j - 1)
                    inap = sc[:].rearrange("p (g w) -> p g w", g=2, w=512)[:, :, 0:392]
                    outap = e[:].rearrange("p (g w) -> p g w", g=2, w=392)
                    nc.scalar.activation(outap, inap, AF.Exp, scale=SCALE)
                    pend = (i, j, e)
            emit_nd(pend)

        # ---- upsample + sigmoid: z = M^T A^T ; up_h = z_h^T A^T ; out = sigmoid ----
        with tc.tile_pool(name="zpsum", bufs=2, space="PSUM") as zp, \
             tc.tile_pool(name="upsum", bufs=2, space="PSUM") as up:
            for i in range(PER):
                zps = zp.tile([28, HO], F32, tag="z", name=f"z{i}")
                nc.tensor.matmul(zps[:], mask28[:, i * 28:(i + 1) * 28], bmat,
                                 start=True, stop=True)
                zs = spool.tile([28, HO], BF16, tag="zs", name=f"zs{i}")
                nc.vector.tensor_copy(out=zs[:], in_=zps[:])
                upp = up.tile([112, 512], F32, tag="up", name=f"up{i}")
                for h in range(2):
                    nc.tensor.matmul(upp[:, h * 224:(h + 1) * 224],
                                     zs[:, h * 112:(h + 1) * 112], bmat,
                                     start=True, stop=True)
                ob = opool.tile([112, 896], F32, tag="ob", name=f"ob{i}")
                nc.scalar.activation(ob[:, 448:896], upp[:, 0:448], AF.Sigmoid, bias=bvap)
                nc.vector.tensor_scalar(ob[:, 0:448], ob[:, 448:896], -1.0, 1.0,
                                        ALU.mult, ALU.add)
                nc.sync.dma_start(
                    out_d[i].rearrange("ch (hh r) c -> r ch hh c", hh=2),
                    ob[:].rearrange("p (ch hh c) -> p ch hh c", ch=2, hh=2))


def _get_nc():
    if "nc" not in _NC_CACHE:
        _NC_CACHE["nc"] = _build_nc()
    return _NC_CACHE["nc"]


def _make_in_maps(x, index, Wq, bq, Wv, bv):
    x = np.asarray(x, np.float32)
    idx = np.asarray(index).astype(np.int64)
    Wq = np.asarray(Wq, np.float32)
    Wv = np.asarray(Wv, np.float32)
    bqv = np.asarray(bq, np.float32).reshape(INTER)
    bvv = float(np.asarray(bv, np.float32).reshape(-1)[0])

    xg = x[idx]
    xb = x.reshape(64, 2, 128, HWN).astype(ml_dtypes.bfloat16)
    xpair = xg.reshape(64, 2, 128, HWN).astype(ml_dtypes.bfloat16)
    wqT = np.ascontiguousarray(Wq.T).reshape(2, 128, INTER)

    cbf0 = np.zeros((128, CBW), np.float32)
    cbf0[:, 0:128] = wqT[0]
    cbf0[:, 128:256] = wqT[1]
    cbf0[0:28, BM0:BM0 + HO] = _bilinear_matrix().T
    cf32 = np.zeros((128, 2), np.float32)
    cf32[:, 0] = bqv
    cf32[0:112, 1] = bvv
    # tiny 1-channel value conv on host (v enters the output linearly)
    v = np.tensordot(Wv[0], xg.reshape(64, 256, HWN), axes=([0], [1]))  # [64, 784]

    in_maps = []
    for c in range(NCORES):
        sl = slice(c * PER, (c + 1) * PER)
        vc = v[sl]  # [8, 784]
        cbf = cbf0.copy()
        for j in range(NJ):
            blk = cbf[0:KT, OV0 + 16 * j:OV0 + 16 * (j + 1)]
            blk[:, 0::2] = 1.0
            blk[:, 1::2] = vc[:, j * KT:(j + 1) * KT].T
        in_maps.append({
            "xin": np.ascontiguousarray(xb[sl]),
            "xpair": np.ascontiguousarray(xpair[sl]),
            "cbf": cbf.astype(ml_dtypes.bfloat16),
            "cf32": cf32,
        })
    return in_maps


def _run(in_maps, trace=False):
    nc = _get_nc()
    res = run_bass_kernel_spmd(nc, in_maps, list(range(NCORES)), trace=trace)
    out = np.concatenate([res.results[c]["out"] for c in range(NCORES)], axis=0)
    return out, res


def kernel(x, lam, index, Wq, bq, Wv, bv, scale_factor):
    in_maps = _make_in_maps(x, index, Wq, bq, Wv, bv)
    out, _ = _run(in_maps)
    return out
